# revision 1
# baseline (speedup 1.0000x reference)
"""Trainium2 Bass kernel for nn_Decoder_22273700397282 (sparse_attention).

Math (per batch b):
    a = concat([h_state, x], -1)                      # (S, 3072)
    bias = h_state.sum(0) @ Ws + ba + bs              # (3072,)
    et = tanh(a @ Wa + bias)                          # (S, 3072)
    attn[s] = softmax_feat(et[s])  if mask[s] else uniform 1/3072
    out = a[trigger] * sum_s attn[s]                  # (3072,)

Implementation notes:
  - Data-parallel over batch: core c owns batches 4c..4c+3. No collectives.
  - Masked rows contribute exactly (1/3072) each (softmax of a constant row),
    so only unmasked rows are computed: rows are compacted on the host and the
    per-batch uniform term n_masked/3072 is added at the end.
  - tanh(z) in [-1,1] makes softmax stable without max-subtraction:
    attn = exp(t) / rowsum(exp(t)).
  - Matmuls in bf16 (PE 4x over fp32).  The per-batch bias row is folded into
    the matmul as two extra one-hot contraction rows (bf16 hi + lo split keeps
    the large bias term at ~f32 accuracy).  K = 3072 + 128 pad = 25 chunks.
  - Row-softmax sum comes free via the activation accum_out; the weighted
    column sum over rows is a PE matmul with lhsT = indicator * (1/rowsum),
    accumulated across row-tiles in a dedicated PSUM region; the indicator
    also encodes batch membership (M=4) and zeroes padding rows.
  - Wa stays resident in SBUF (25 x (128,3072) bf16 = 150KB/partition).
"""
import math

import numpy as np
import ml_dtypes

import concourse.bacc as bacc
import concourse.tile as tile
import concourse.mybir as mybir
from concourse import bass_utils

BF16 = mybir.dt.bfloat16
F32 = mybir.dt.float32
AFT = mybir.ActivationFunctionType
BF = ml_dtypes.bfloat16

B, S, IN = 32, 512, 1024
D = 3 * IN            # 3072 features / out size
KD = 2 * IN           # 2048 h_state features
KC = 25               # contraction chunks of 128 (3072 data + 128 bias/pad)
NB = 4                # batches per core
NCORES = 8
NCH = D // 512        # 6 output chunks of 512

LAST_EXEC_NS = None
_PROG_CACHE = {}


def _build_program(T):
    """Bass program for T row-tiles of 128 compacted rows per core."""
    nc = bacc.Bacc("TRN2", target_bir_lowering=False, debug=False)
    at_h = nc.dram_tensor("at", [T, 128, KC * 128], BF16, kind="ExternalInput")
    wa_h = nc.dram_tensor("wa", [KC, 128, D], BF16, kind="ExternalInput")
    ind_h = nc.dram_tensor("ind", [128, T * NB], BF16, kind="ExternalInput")
    trig_h = nc.dram_tensor("trig", [NB, D], F32, kind="ExternalInput")
    u_h = nc.dram_tensor("u", [NB, 1], F32, kind="ExternalInput")
    out_h = nc.dram_tensor("out", [NB, D], F32, kind="ExternalOutput")

    with tile.TileContext(nc) as tc:
        with (
            tc.tile_pool(name="wa_pool", bufs=1) as wa_pool,
            tc.tile_pool(name="at_pool", bufs=2) as at_pool,
            tc.tile_pool(name="small", bufs=2) as small,
            tc.tile_pool(name="epool", bufs=2) as epool,
            tc.tile_pool(name="psum_main", bufs=2, space="PSUM") as psum_main,
            tc.tile_pool(name="psum_acc", bufs=1, space="PSUM") as psum_acc,
        ):
            wa = wa_pool.tile([128, KC * D], BF16)
            for k in range(KC):
                nc.sync.dma_start(wa[:, k * D:(k + 1) * D], wa_h[k])
            ind_all = wa_pool.tile([128, T * NB], BF16)
            nc.sync.dma_start(ind_all[:], ind_h[:])

            psA = psum_acc.tile([NB, D], F32)

            for t in range(T):
                at = at_pool.tile([128, KC * 128], BF16)
                nc.sync.dma_start(at[:], at_h[t])
                et = epool.tile([128, D], BF16)
                rp = small.tile([128, NCH], F32)
                for ni in range(NCH):
                    ps = psum_main.tile([128, 512], F32)
                    for k in range(KC):
                        nc.tensor.matmul(
                            ps[:],
                            at[:, k * 128:(k + 1) * 128],
                            wa[:, k * D + ni * 512: k * D + ni * 512 + 512],
                            start=(k == 0),
                            stop=(k == KC - 1),
                        )
                    tt = small.tile([128, 512], BF16)
                    nc.scalar.activation(tt[:], ps[:], AFT.Tanh)
                    nc.scalar.activation(
                        et[:, ni * 512:(ni + 1) * 512], tt[:], AFT.Exp,
                        accum_out=rp[:, ni:ni + 1],
                    )
                r = small.tile([128, 1], F32)
                nc.vector.tensor_reduce(
                    r[:], rp[:], mybir.AxisListType.X, mybir.AluOpType.add)
                rinv = small.tile([128, 1], F32)
                nc.vector.reciprocal(rinv[:], r[:])
                lhsT4 = small.tile([128, NB], BF16)
                nc.vector.tensor_scalar_mul(
                    lhsT4[:], ind_all[:, t * NB:(t + 1) * NB], rinv[:])
                for ni in range(NCH):
                    nc.tensor.matmul(
                        psA[:, ni * 512:(ni + 1) * 512],
                        lhsT4[:],
                        et[:, ni * 512:(ni + 1) * 512],
                        start=(t == 0),
                        stop=(t == T - 1),
                    )

            for ni in range(NCH):
                sl = slice(ni * 512, (ni + 1) * 512)
                trign = small.tile([NB, 512], F32)
                nc.sync.dma_start(trign[:], trig_h[:, sl])
                un = small.tile([NB, 1], F32)
                nc.sync.dma_start(un[:], u_h[:])
                accn = small.tile([NB, 512], F32)
                nc.vector.tensor_scalar_add(accn[:], psA[:, sl], un[:])
                outn = small.tile([NB, 512], F32)
                nc.vector.tensor_mul(outn[:], accn[:], trign[:])
                nc.sync.dma_start(out_h[:, sl], outn[:])
    nc.compile()
    return nc


def kernel(h_state, x, trigger, mask, Wa, ba, Ws, bs, *, trace=False):
    global LAST_EXEC_NS
    h_state = np.asarray(h_state, dtype=np.float32)
    x = np.asarray(x, dtype=np.float32)
    trigger = np.asarray(trigger).astype(np.int64)
    mask = np.asarray(mask)
    Wa = np.asarray(Wa, dtype=np.float32)
    ba = np.asarray(ba, dtype=np.float32)
    Ws = np.asarray(Ws, dtype=np.float32)
    bs = np.asarray(bs, dtype=np.float32)

    # per-batch bias row (f64 for accuracy; dominates z's magnitude)
    s_sum = h_state.sum(axis=1, dtype=np.float64)                  # (B, 2048)
    bias = (s_sum @ Ws.astype(np.float64)
            + ba.astype(np.float64) + bs.astype(np.float64)).astype(np.float32)
    bias_hi = bias.astype(BF)
    bias_lo = (bias - bias_hi.astype(np.float32)).astype(BF)       # (B, D)

    # trigger rows of a = [h_state | x]
    bi = np.arange(B)
    trig_full = np.concatenate(
        [h_state[bi, trigger], x[bi, trigger]], axis=1)            # (B, D)

    keep = [np.flatnonzero(np.asarray(mask[b]) != 0) for b in range(B)]
    n_rows_core = [
        sum(len(keep[c * NB + j]) for j in range(NB)) for c in range(NCORES)]
    T = max(1, max(math.ceil(r / 128) for r in n_rows_core))

    # shared bf16 weight block: rows 0..3071 = Wa, rest zero (bias rows set
    # per-core below)
    wa_base = np.zeros((KC * 128, D), dtype=BF)
    wa_base[:D] = Wa.astype(BF)

    in_maps = []
    for c in range(NCORES):
        rows_h = []           # compacted h_state rows
        rows_x = []           # compacted x rows
        owner = []            # batch-within-core per row
        for j in range(NB):
            b = c * NB + j
            idx = keep[b]
            rows_h.append(h_state[b, idx])
            rows_x.append(x[b, idx])
            owner.append(np.full(len(idx), j, dtype=np.int64))
        rows_h = np.concatenate(rows_h, axis=0)
        rows_x = np.concatenate(rows_x, axis=0)
        owner = np.concatenate(owner, axis=0)
        rc = rows_h.shape[0]

        a_c = np.zeros((T * 128, KC * 128), dtype=np.float32)
        a_c[:rc, :KD] = rows_h
        a_c[:rc, KD:D] = rows_x
        a_c[np.arange(rc), D + 2 * owner] = 1.0       # bias hi one-hot
        a_c[np.arange(rc), D + 2 * owner + 1] = 1.0   # bias lo one-hot
        att = np.ascontiguousarray(
            a_c.astype(BF).reshape(T, 128, KC, 128).transpose(0, 3, 2, 1)
        ).reshape(T, 128, KC * 128)

        wae = wa_base.copy()
        for j in range(NB):
            b = c * NB + j
            wae[D + 2 * j] = bias_hi[b]
            wae[D + 2 * j + 1] = bias_lo[b]
        wae = np.ascontiguousarray(wae.reshape(KC, 128, D))

        ind_all = np.zeros((128, T * NB), dtype=BF)
        r_idx = np.arange(rc)
        ind_all[r_idx % 128, (r_idx // 128) * NB + owner] = 1.0

        trig = np.ascontiguousarray(trig_full[c * NB:(c + 1) * NB])
        u = np.array(
            [[(S - len(keep[c * NB + j])) / np.float32(D)] for j in range(NB)],
            dtype=np.float32)
        in_maps.append({"at": att, "wa": wae, "ind": ind_all,
                        "trig": trig, "u": u})

    if T not in _PROG_CACHE:
        _PROG_CACHE[T] = _build_program(T)
    nc = _PROG_CACHE[T]

    res = bass_utils.run_bass_kernel_spmd(
        nc, in_maps, list(range(NCORES)), trace=trace)
    LAST_EXEC_NS = res.exec_time_ns
    return np.concatenate(
        [np.asarray(res.results[c]["out"]) for c in range(NCORES)], axis=0)


# revision 2
# speedup vs baseline: 1.0846x; 1.0846x over previous
"""Trainium2 Bass kernel for nn_Decoder_22273700397282 (sparse_attention).

Math (per batch b):
    a = concat([h_state, x], -1)                      # (S, 3072)
    bias = h_state.sum(0) @ Ws + ba + bs              # (3072,)
    et = tanh(a @ Wa + bias)                          # (S, 3072)
    attn[s] = softmax_feat(et[s])  if mask[s] else uniform 1/3072
    out = a[trigger] * sum_s attn[s]                  # (3072,)

Implementation notes:
  - Data-parallel over batch: core c owns batches 4c..4c+3. No collectives.
  - Masked rows contribute exactly (1/3072) each (softmax of a constant row),
    so only unmasked rows are computed: rows are compacted on the host and the
    per-batch uniform term n_masked/3072 is added at the end.
  - tanh(z) in [-1,1] makes softmax stable without max-subtraction:
    attn = exp(t) / rowsum(exp(t)).
  - Matmuls in bf16 (PE 4x over fp32).  The per-batch bias row is folded into
    the matmul as two extra one-hot contraction rows (bf16 hi + lo split keeps
    the large bias term at ~f32 accuracy).  K = 3072 + 128 pad = 25 chunks.
  - Row-softmax sum comes free via the activation accum_out; the weighted
    column sum over rows is a PE matmul with lhsT = indicator * (1/rowsum),
    accumulated across row-tiles in a dedicated PSUM region; the indicator
    also encodes batch membership (M=4) and zeroes padding rows.
  - Wa stays resident in SBUF (25 x (128,3072) bf16 = 150KB/partition).
"""
import math

import numpy as np
import ml_dtypes

import concourse.bacc as bacc
import concourse.tile as tile
import concourse.mybir as mybir
from concourse import bass_utils

BF16 = mybir.dt.bfloat16
F32 = mybir.dt.float32
AFT = mybir.ActivationFunctionType
BF = ml_dtypes.bfloat16

B, S, IN = 32, 512, 1024
D = 3 * IN            # 3072 features / out size
KD = 2 * IN           # 2048 h_state features
KC = 25               # contraction chunks of 128 (3072 data + 128 bias/pad)
NB = 4                # batches per core
NCORES = 8
NCH = D // 512        # 6 output chunks of 512

LAST_EXEC_NS = None
_PROG_CACHE = {}


def _build_program(T):
    """Bass program for T row-tiles of 128 compacted rows per core."""
    nc = bacc.Bacc("TRN2", target_bir_lowering=False, debug=False)
    at_h = nc.dram_tensor("at", [T, 128, KC * 128], BF16, kind="ExternalInput")
    wa_h = nc.dram_tensor("wa", [KC, 128, D], BF16, kind="ExternalInput")
    ind_h = nc.dram_tensor("ind", [128, T * NB], BF16, kind="ExternalInput")
    trig_h = nc.dram_tensor("trig", [NB, D], F32, kind="ExternalInput")
    u_h = nc.dram_tensor("u", [NB, 1], F32, kind="ExternalInput")
    out_h = nc.dram_tensor("out", [NB, D], F32, kind="ExternalOutput")

    with tile.TileContext(nc) as tc:
        with (
            tc.tile_pool(name="wa_pool", bufs=1) as wa_pool,
            tc.tile_pool(name="at_pool", bufs=2) as at_pool,
            tc.tile_pool(name="small", bufs=2) as small,
            tc.tile_pool(name="epool", bufs=2) as epool,
        ):
            # tile 0's lhsT first so PE can start as soon as wa chunk 0 lands
            at0 = at_pool.tile([128, KC * 128], BF16, tag="at")
            nc.sync.dma_start(at0[:], at_h[0])
            wa = wa_pool.tile([128, KC * D], BF16)
            for k in range(KC):
                nc.sync.dma_start(wa[:, k * D:(k + 1) * D], wa_h[k])
            ind_all = wa_pool.tile([128, T * NB], BF16)
            nc.sync.dma_start(ind_all[:], ind_h[:])
            trig_sb = wa_pool.tile([NB, D], F32)
            nc.sync.dma_start(trig_sb[:], trig_h[:])
            u_sb = wa_pool.tile([NB, 1], F32)
            nc.sync.dma_start(u_sb[:], u_h[:])

            def softmax_tail(t, et, rp):
                """row-sum -> 1/r -> batch-indicator lhsT for the column sum"""
                r = small.tile([128, 1], F32)
                nc.vector.tensor_reduce(
                    r[:], rp[:], mybir.AxisListType.X, mybir.AluOpType.add)
                rinv = small.tile([128, 1], F32)
                nc.vector.reciprocal(rinv[:], r[:])
                lhsT4 = small.tile([128, NB], BF16)
                nc.vector.tensor_scalar_mul(
                    lhsT4[:], ind_all[:, t * NB:(t + 1) * NB], rinv[:])
                return lhsT4

            # ---- phase 1: tile 0, k-outer so PE paces with the Wa stream ----
            et0 = epool.tile([128, D], BF16, tag="et")
            rp0 = small.tile([128, NCH], F32, tag="rp")
            with tc.tile_pool(name="psum_p1", bufs=1, space="PSUM") as psum_p1:
                ps6 = psum_p1.tile([128, NCH * 512], F32)
                for k in range(KC):
                    for ni in range(NCH):
                        nc.tensor.matmul(
                            ps6[:, ni * 512:(ni + 1) * 512],
                            at0[:, k * 128:(k + 1) * 128],
                            wa[:, k * D + ni * 512: k * D + ni * 512 + 512],
                            start=(k == 0),
                            stop=(k == KC - 1),
                        )
                for ni in range(NCH):
                    tt = small.tile([128, 512], BF16, tag="tt")
                    nc.scalar.activation(
                        tt[:], ps6[:, ni * 512:(ni + 1) * 512], AFT.Tanh)
                    nc.scalar.activation(
                        et0[:, ni * 512:(ni + 1) * 512], tt[:], AFT.Exp,
                        accum_out=rp0[:, ni:ni + 1],
                    )
            lhsT4_0 = softmax_tail(0, et0, rp0)

            # ---- phase 2: steady state ----
            with (
                tc.tile_pool(name="psum_main", bufs=2, space="PSUM") as psum_main,
                tc.tile_pool(name="psum_acc", bufs=1, space="PSUM") as psum_acc,
            ):
                psA = psum_acc.tile([NB, D], F32)
                for ni in range(NCH):
                    nc.tensor.matmul(
                        psA[:, ni * 512:(ni + 1) * 512],
                        lhsT4_0[:],
                        et0[:, ni * 512:(ni + 1) * 512],
                        start=True, stop=(T == 1),
                    )

                for t in range(1, T):
                    at = at_pool.tile([128, KC * 128], BF16, tag="at")
                    nc.sync.dma_start(at[:], at_h[t])
                    et = epool.tile([128, D], BF16, tag="et")
                    rp = small.tile([128, NCH], F32, tag="rp")
                    for ni in range(NCH):
                        ps = psum_main.tile([128, 512], F32)
                        for k in range(KC):
                            nc.tensor.matmul(
                                ps[:],
                                at[:, k * 128:(k + 1) * 128],
                                wa[:, k * D + ni * 512: k * D + ni * 512 + 512],
                                start=(k == 0),
                                stop=(k == KC - 1),
                            )
                        tt = small.tile([128, 512], BF16, tag="tt")
                        nc.scalar.activation(tt[:], ps[:], AFT.Tanh)
                        nc.scalar.activation(
                            et[:, ni * 512:(ni + 1) * 512], tt[:], AFT.Exp,
                            accum_out=rp[:, ni:ni + 1],
                        )
                    lhsT4 = softmax_tail(t, et, rp)
                    for ni in range(NCH):
                        nc.tensor.matmul(
                            psA[:, ni * 512:(ni + 1) * 512],
                            lhsT4[:],
                            et[:, ni * 512:(ni + 1) * 512],
                            start=False, stop=(t == T - 1),
                        )

                for ni in range(NCH):
                    sl = slice(ni * 512, (ni + 1) * 512)
                    accn = small.tile([NB, 512], F32)
                    nc.vector.tensor_scalar_add(accn[:], psA[:, sl], u_sb[:])
                    outn = small.tile([NB, 512], F32)
                    nc.vector.tensor_mul(outn[:], accn[:], trig_sb[:, sl])
                    nc.sync.dma_start(out_h[:, sl], outn[:])
    nc.compile()
    return nc


def kernel(h_state, x, trigger, mask, Wa, ba, Ws, bs, *, trace=False):
    global LAST_EXEC_NS
    h_state = np.asarray(h_state, dtype=np.float32)
    x = np.asarray(x, dtype=np.float32)
    trigger = np.asarray(trigger).astype(np.int64)
    mask = np.asarray(mask)
    Wa = np.asarray(Wa, dtype=np.float32)
    ba = np.asarray(ba, dtype=np.float32)
    Ws = np.asarray(Ws, dtype=np.float32)
    bs = np.asarray(bs, dtype=np.float32)

    # per-batch bias row (f64 for accuracy; dominates z's magnitude)
    s_sum = h_state.sum(axis=1, dtype=np.float64)                  # (B, 2048)
    bias = (s_sum @ Ws.astype(np.float64)
            + ba.astype(np.float64) + bs.astype(np.float64)).astype(np.float32)
    bias_hi = bias.astype(BF)
    bias_lo = (bias - bias_hi.astype(np.float32)).astype(BF)       # (B, D)

    # trigger rows of a = [h_state | x]
    bi = np.arange(B)
    trig_full = np.concatenate(
        [h_state[bi, trigger], x[bi, trigger]], axis=1)            # (B, D)

    keep = [np.flatnonzero(np.asarray(mask[b]) != 0) for b in range(B)]
    n_rows_core = [
        sum(len(keep[c * NB + j]) for j in range(NB)) for c in range(NCORES)]
    T = max(1, max(math.ceil(r / 128) for r in n_rows_core))

    # shared bf16 weight block: rows 0..3071 = Wa, rest zero (bias rows set
    # per-core below)
    wa_base = np.zeros((KC * 128, D), dtype=BF)
    wa_base[:D] = Wa.astype(BF)

    in_maps = []
    for c in range(NCORES):
        rows_h = []           # compacted h_state rows
        rows_x = []           # compacted x rows
        owner = []            # batch-within-core per row
        for j in range(NB):
            b = c * NB + j
            idx = keep[b]
            rows_h.append(h_state[b, idx])
            rows_x.append(x[b, idx])
            owner.append(np.full(len(idx), j, dtype=np.int64))
        rows_h = np.concatenate(rows_h, axis=0)
        rows_x = np.concatenate(rows_x, axis=0)
        owner = np.concatenate(owner, axis=0)
        rc = rows_h.shape[0]

        a_c = np.zeros((T * 128, KC * 128), dtype=np.float32)
        a_c[:rc, :KD] = rows_h
        a_c[:rc, KD:D] = rows_x
        a_c[np.arange(rc), D + 2 * owner] = 1.0       # bias hi one-hot
        a_c[np.arange(rc), D + 2 * owner + 1] = 1.0   # bias lo one-hot
        att = np.ascontiguousarray(
            a_c.astype(BF).reshape(T, 128, KC, 128).transpose(0, 3, 2, 1)
        ).reshape(T, 128, KC * 128)

        wae = wa_base.copy()
        for j in range(NB):
            b = c * NB + j
            wae[D + 2 * j] = bias_hi[b]
            wae[D + 2 * j + 1] = bias_lo[b]
        wae = np.ascontiguousarray(wae.reshape(KC, 128, D))

        ind_all = np.zeros((128, T * NB), dtype=BF)
        r_idx = np.arange(rc)
        ind_all[r_idx % 128, (r_idx // 128) * NB + owner] = 1.0

        trig = np.ascontiguousarray(trig_full[c * NB:(c + 1) * NB])
        u = np.array(
            [[(S - len(keep[c * NB + j])) / np.float32(D)] for j in range(NB)],
            dtype=np.float32)
        in_maps.append({"at": att, "wa": wae, "ind": ind_all,
                        "trig": trig, "u": u})

    if T not in _PROG_CACHE:
        _PROG_CACHE[T] = _build_program(T)
    nc = _PROG_CACHE[T]

    res = bass_utils.run_bass_kernel_spmd(
        nc, in_maps, list(range(NCORES)), trace=trace)
    LAST_EXEC_NS = res.exec_time_ns
    return np.concatenate(
        [np.asarray(res.results[c]["out"]) for c in range(NCORES)], axis=0)
